# revision 14
# baseline (speedup 1.0000x reference)
"""3-layer GAT (ogbn-arxiv shapes) on 8 Trainium2 NeuronCores.

Graph/data-parallel per the sharding hint: nodes are sharded contiguously
across the 8 cores.  Per layer each core projects its shard with an augmented
weight matrix (attention vectors ride as extra columns), AllGathers the
augmented node table, then aggregates its incoming edges: int16 dma_gather of
source rows (6 static 32768-row ranges of the table), per-edge softmax weights
on ACT/DVE, and segment-sums via one-hot matrices on the tensor engine into
per-tile PSUM accumulators.  BN+ReLU and the next layer's projection are fused
into the per-tile finalize; log_softmax at the end.

Edge->slot layout is computed on the host with a static chunk->tile-pair
assignment so the SPMD program is identical on every core.
"""

import numpy as np

import concourse.bacc as bacc
import concourse.bass as bass
import concourse.mybir as mybir
import concourse.tile as tile
from concourse.bass_utils import run_bass_kernel_spmd

F32 = mybir.dt.float32
I16 = mybir.dt.int16
AF = mybir.ActivationFunctionType
OP = mybir.AluOpType

NCORES = 8

# L1 row: [h0f 128 | one | h1f 128 | one | als0 als1 | ald0 ald1 | 0...] w=320
# L2 row: [feats 256 | one | als | ald | 0...] w=320
# L3 row: [feats 40 | one | als | ald | 0...] w=64
LAYERS = [
    dict(TW=320, H=2, RW=129, ALS=258, ATT0=256, ALDB=4),
    dict(TW=320, H=1, RW=257, ALS=257, ATT0=256, ALDB=2),
    dict(TW=64, H=1, RW=41, ALS=41, ATT0=0, ALDB=42),
]


def make_cfg(n_nodes, tiles_per_core, n_ranges):
    nsh = tiles_per_core * 128
    cfg = dict(N=n_nodes, NSH=nsh, TILES=tiles_per_core,
               PAIRS=(tiles_per_core + 1) // 2, NR=n_ranges, NTOT=nsh * NCORES)
    cfg["RANGE"] = -(-cfg["NTOT"] // n_ranges)
    assert cfg["RANGE"] <= 32768
    cfg["GROUPS"] = -(-cfg["PAIRS"] // 3)
    return cfg


def group_pairs(cfg, g):
    return [p for p in range(3 * g, 3 * g + 3) if p < cfg["PAIRS"]]


def cfg_slots(cfg):
    return sum(cfg["NR"] * 256 * len(group_pairs(cfg, g)) for g in range(cfg["GROUPS"]))


# ------------------------------------------------------------------ host prep


def prepare(cfg, x, src, dst):
    N, NSH, NTOT, NR = cfg["N"], cfg["NSH"], cfg["NTOT"], cfg["NR"]
    s = src.astype(np.int64)
    d = dst.astype(np.int64)

    perm = np.arange(NTOT, dtype=np.int64)  # node -> row
    for _ in range(30):
        srow, drow = perm[s], perm[d]
        key = ((drow // NSH) * cfg["PAIRS"] + (drow % NSH) // 256) * NR + srow // cfg["RANGE"]
        cnt = np.bincount(key, minlength=NCORES * cfg["PAIRS"] * NR)
        over = np.nonzero(cnt > 256)[0]
        if len(over) == 0:
            break
        inv = np.empty(NTOT, np.int64)
        inv[perm] = np.arange(NTOT)
        cnt3 = cnt.reshape(NCORES, cfg["PAIRS"], NR)
        load = cnt3.max(axis=2)
        for k in over[:64]:
            c = k // (cfg["PAIRS"] * NR)
            p = (k // NR) % cfg["PAIRS"]
            p2 = int(np.argmin(load[c]))
            a = inv[c * NSH + p * 256]
            b = inv[c * NSH + p2 * 256]
            perm[a], perm[b] = perm[b], perm[a]
            load[c, p2] += 8
    else:
        raise RuntimeError("pair balancing failed")

    srow, drow = perm[s], perm[d]
    core = drow // NSH
    pair = (drow % NSH) // 256
    rng = srow // cfg["RANGE"]

    G = cfg["GROUPS"]
    blk_off = np.zeros(cfg["PAIRS"], np.int64)
    call_off = np.zeros((G, NR), np.int64)
    off = 0
    for g in range(G):
        bp = group_pairs(cfg, g)
        for j, p in enumerate(bp):
            blk_off[p] = 256 * j
        for r in range(NR):
            call_off[g, r] = off
            off += 256 * len(bp)
    slots = off
    assert slots == cfg_slots(cfg)

    gidx = np.zeros((NCORES, slots), np.int16)
    didx = np.zeros((NCORES, slots), np.int16)
    dstloc = np.full((NCORES, slots), -1.0, np.float32)

    order = np.lexsort((drow, rng, pair, core))
    so, do_, co, po, ro = srow[order], drow[order], core[order], pair[order], rng[order]
    keyo = (co * cfg["PAIRS"] + po) * NR + ro
    chg = np.empty(len(keyo), bool)
    if len(keyo) == 0:
        chg = chg
    else:
        chg[0] = True
    chg[1:] = keyo[1:] != keyo[:-1]
    if len(keyo):
        sidx = np.nonzero(chg)[0]
        pos = np.arange(len(keyo)) - np.repeat(sidx, np.diff(np.append(sidx, len(keyo))))
        assert pos.max() < 256
        sl = call_off[po // 3, ro] + blk_off[po] + pos
        gidx[co, sl] = (so - ro * cfg["RANGE"]).astype(np.int16)
        didx[co, sl] = (do_ - co * NSH).astype(np.int16)
        dstloc[co, sl] = (do_ - (co * NSH + po * 256)).astype(np.float32)

    def wrap(a):  # index i -> [i%16, i//16], replicated to 128 partitions
        ncol = a.shape[1] // 16
        w = a.reshape(a.shape[0], ncol, 16).transpose(0, 2, 1)
        return np.ascontiguousarray(np.tile(w, (1, 8, 1)))

    nch = slots // 128
    dstloc_c = np.ascontiguousarray(dstloc.reshape(NCORES, nch, 128).transpose(0, 2, 1))

    inv = np.empty(NTOT, np.int64)
    inv[perm] = np.arange(NTOT)
    xsh = np.zeros((NCORES, NSH, x.shape[1]), np.float32)
    for c in range(NCORES):
        rows = inv[c * NSH : (c + 1) * NSH]
        real = rows < N
        xsh[c][real] = x[rows[real]]
    xT = np.ascontiguousarray(xsh.transpose(0, 2, 1))
    return wrap(gidx), wrap(didx), dstloc_c, xT, perm


def prep_weights(inp):
    EPS = 1e-5
    HID = inp["as1"].shape[1]
    w1 = np.zeros((inp["W1"].shape[0], 320), np.float32)
    w1[:, 0:128] = inp["W1"][:, :HID]
    w1[:, 129:257] = inp["W1"][:, HID:]
    w1[:, 258] = inp["W1"][:, :HID] @ inp["as1"][0]
    w1[:, 259] = inp["W1"][:, HID:] @ inp["as1"][1]
    w1[:, 260] = inp["W1"][:, :HID] @ inp["ad1"][0]
    w1[:, 261] = inp["W1"][:, HID:] @ inp["ad1"][1]
    w2 = np.zeros((256, 320), np.float32)
    w2[:, 0:256] = inp["W2"]
    w2[:, 257] = inp["W2"] @ inp["as2"][0]
    w2[:, 258] = inp["W2"] @ inp["ad2"][0]
    w3 = np.zeros((256, 64), np.float32)
    w3[:, 0:40] = inp["W3"]
    w3[:, 41] = inp["W3"] @ inp["as3"][0]
    w3[:, 42] = inp["W3"] @ inp["ad3"][0]

    def fold(b, g, be, m, v):
        k = g / np.sqrt(v + EPS)
        return k.astype(np.float32), ((b - m) * k + be).astype(np.float32)

    A1, B1 = fold(inp["b1"], inp["g1"], inp["be1"], inp["m1"], inp["v1"])
    A2, B2 = fold(inp["b2"], inp["g2"], inp["be2"], inp["m2"], inp["v2"])
    rep = lambda a: np.ascontiguousarray(np.tile(a[None, :], (128, 1)))
    return dict(w1=w1, w2=w2, w3=w3, A1=rep(A1), B1=rep(B1), A2=rep(A2),
                B2=rep(B2), b3=rep(inp["b3"].astype(np.float32)),
                iota=rep(np.arange(256, dtype=np.float32)),
                pidx=np.ascontiguousarray(
                    np.arange(128, dtype=np.float32)[:, None]),
                ident=np.eye(128, dtype=np.float32))


# ------------------------------------------------------------------ builder


def build(cfg):
    NSH, TILES, NR, G = cfg["NSH"], cfg["TILES"], cfg["NR"], cfg["GROUPS"]
    SLOTS = cfg_slots(cfg)
    nc = bacc.Bacc()
    ext = lambda n, sh, dt=F32: nc.dram_tensor(n, sh, dt, kind="ExternalInput")
    D = dict(
        xT=ext("xT", [128, NSH]), w1=ext("w1", [128, 320]),
        w2=ext("w2", [256, 320]), w3=ext("w3", [256, 64]),
        A1=ext("A1", [128, 256]), B1=ext("B1", [128, 256]),
        A2=ext("A2", [128, 256]), B2=ext("B2", [128, 256]),
        b3=ext("b3", [128, 40]), iota=ext("iota", [128, 256]),
        pidx=ext("pidx", [128, 1]),
        ident=ext("ident", [128, 128]),
        gidx=ext("gidx", [128, SLOTS // 16], I16),
        didx=ext("didx", [128, SLOTS // 16], I16),
        dstloc=ext("dstloc", [128, SLOTS // 128]),
    )
    out = nc.dram_tensor("out", [NSH, 40], F32, kind="ExternalOutput")
    dbgG = nc.dram_tensor("dbgG", [128, 6 * 320], F32, kind="ExternalOutput")
    haug = [nc.dram_tensor(f"haug{l}", [NSH, LAYERS[l]["TW"]], F32) for l in range(3)]
    tabs = [nc.dram_tensor(f"tab{l}", [cfg["NTOT"], LAYERS[l]["TW"]], F32,
                           addr_space="Shared") for l in range(3)]

    with tile.TileContext(nc) as tc:
        with (
            tc.tile_pool(name="res", bufs=1) as res,
            tc.tile_pool(name="gp", bufs=2) as gp,
            tc.tile_pool(name="wp", bufs=3) as wp,
            tc.tile_pool(name="pt", bufs=1, space="PSUM") as pt,
            tc.tile_pool(name="pz", bufs=1, space="PSUM") as pz,
            tc.tile_pool(name="pagg", bufs=1, space="PSUM") as pagg,
        ):
            R = {}
            for n, sh, dt in (
                ("w1", [128, 320], F32), ("w2", [128, 640], F32),
                ("w3", [128, 128], F32), ("A1", [128, 256], F32),
                ("B1", [128, 256], F32), ("A2", [128, 256], F32),
                ("B2", [128, 256], F32), ("b3", [128, 40], F32),
                ("iota", [128, 256], F32), ("ident", [128, 128], F32),
                ("pidx", [128, 1], F32),
                ("gidx", [128, SLOTS // 16], I16),
                ("didx", [128, SLOTS // 16], I16),
                ("dstloc", [128, SLOTS // 128], F32),
            ):
                R[n] = res.tile(sh, dt, name=n, tag=n)
                if n in ("w2", "w3"):
                    w = sh[1] // 2
                    for k in range(2):
                        nc.scalar.dma_start(
                            out=R[n][:, k * w : (k + 1) * w],
                            in_=D[n][k * 128 : (k + 1) * 128, :])
                else:
                    nc.scalar.dma_start(out=R[n][:], in_=D[n][:])

            # ---- layer-1 projection
            for t in range(TILES):
                xt = wp.tile([128, 128], F32, tag="xt")
                nc.scalar.dma_start(out=xt[:], in_=D["xT"][:, t * 128 : (t + 1) * 128])
                ps = pz.tile([128, 320], F32, tag="proj")
                nc.tensor.matmul(ps[:], lhsT=xt[:], rhs=R["w1"][:], start=True, stop=True)
                hs = wp.tile([128, 320], F32, tag="hs")
                nc.scalar.activation(hs[:], ps[:], AF.Copy)
                nc.vector.memset(hs[:, 128:129], 1.0)
                nc.vector.memset(hs[:, 257:258], 1.0)
                nc.scalar.dma_start(out=haug[0][t * 128 : (t + 1) * 128, :], in_=hs[:])

            for l in range(3):
                nc.gpsimd.collective_compute(
                    "AllGather", OP.bypass,
                    ins=[haug[l][:].opt()], outs=[tabs[l][:].opt()],
                    replica_groups=[list(range(NCORES))])
                edge_phase(nc, cfg, l, R, out, haug, tabs, gp, wp, pt, pz, pagg, dbgG)
    nc.compile()
    return nc


def edge_phase(nc, cfg, l, R, out, haug, tabs, gp, wp, pt, pz, pagg, dbgG=None):
    L = LAYERS[l]
    TW, H, RW = L["TW"], L["H"], L["RW"]
    NR, G, TILES = cfg["NR"], cfg["GROUPS"], cfg["TILES"]
    PW = H * RW  # psum width used
    call16 = 0
    ccol0 = 0
    for g in range(G):
        pairs = group_pairs(cfg, g)
        ns = 256 * len(pairs)
        nb = ns // 128
        # gathers
        Gt = []
        for r in range(NR):
            gt = gp.tile([128, 6 * TW], F32, tag=f"G{r}")
            lo = r * cfg["RANGE"]
            hi = min(lo + cfg["RANGE"], cfg["NTOT"])
            nc.gpsimd.dma_gather(
                out_ap=gt[:, : nb * TW].rearrange("p (b t) -> p b t", b=nb),
                in_ap=tabs[l][lo:hi, :],
                idxs_ap=R["gidx"][:, call16 + r * (ns // 16) : call16 + (r + 1) * (ns // 16)],
                num_idxs=ns, num_idxs_reg=ns, elem_size=TW, single_packet=False)
            if l == 0 and g == 0 and r == 0 and dbgG is not None:
                nc.scalar.dma_start(out=dbgG[:, : nb * TW], in_=gt[:, : nb * TW])
            Gt.append(gt)
        ad = gp.tile([128, 6 * NR * 64], F32, tag="ald")
        nc.gpsimd.dma_gather(
            out_ap=ad[:, : NR * nb * 64].rearrange("p (b t) -> p b t", b=NR * nb),
            in_ap=haug[l][:, L["ATT0"] : L["ATT0"] + 64],
            idxs_ap=R["didx"][:, call16 : call16 + NR * (ns // 16)],
            num_idxs=NR * ns, num_idxs_reg=NR * ns, elem_size=64,
            elem_step=TW, single_packet=False)
        # per-edge weights exp(lrelu(als[src] + ald[dst]))
        exw = []
        for r in range(NR):
            ex = wp.tile([128, 12], F32, tag=f"ex{r}")
            gv = Gt[r][:, : nb * TW].rearrange("p (b t) -> p b t", b=nb)
            av = ad[:, : NR * nb * 64].rearrange("p (b t) -> p b t", b=NR * nb)
            ev = ex[:, : nb * H].rearrange("p (b t) -> p b t", b=nb)
            nc.vector.tensor_tensor(
                out=ev, in0=gv[:, :, L["ALS"] : L["ALS"] + H],
                in1=av[:, r * nb : (r + 1) * nb, L["ALDB"] : L["ALDB"] + H],
                op=OP.add)
            ex2 = wp.tile([128, 12], F32, tag=f"ex2_{r}")
            nc.vector.tensor_scalar(out=ex2[:, : nb * H], in0=ex[:, : nb * H],
                                    scalar1=0.2, scalar2=None, op0=OP.mult)
            nc.vector.tensor_tensor(out=ex[:, : nb * H], in0=ex[:, : nb * H],
                                    in1=ex2[:, : nb * H], op=OP.max)
            nc.scalar.activation(ex[:, : nb * H], ex[:, : nb * H], AF.Exp)
            if l == 0 and g == 0 and r == 0 and dbgG is not None:
                nc.scalar.dma_start(out=dbgG[:, 1600 : 1600 + nb * H], in_=ex[:, : nb * H])
            exw.append(ex)
        # chunk matmuls into per-tile psums
        ptile = {}
        for jp, p in enumerate(pairs):
            for side in range(2):
                t = 2 * p + side
                if t < TILES:
                    ptile[t] = pagg.tile([128, 272], F32, name=f"agg_t{t}", tag=f"agg{t % 6}")
        started = set()
        for jp, p in enumerate(pairs):
            for r in range(NR):
                for side in range(2):
                    b = 2 * jp + side
                    ccol = ccol0 + r * nb + b
                    for h in range(H):
                        s2 = wp.tile([128, 256], F32, tag="s2")
                        nc.vector.tensor_scalar(
                            out=s2[:], in0=R["iota"][:],
                            scalar1=R["dstloc"][:, ccol : ccol + 1],
                            scalar2=exw[r][:, b * H + h : b * H + h + 1],
                            op0=OP.is_equal, op1=OP.mult)
                        if l == 0:
                            rhs = Gt[r][:, b * TW + h * 129 : b * TW + h * 129 + RW]
                        else:
                            rhs = Gt[r][:, b * TW : b * TW + RW]
                        for ti in range(2):
                            t = 2 * p + ti
                            if t >= TILES:
                                continue
                            nc.tensor.matmul(
                                ptile[t][:, h * RW : (h + 1) * RW],
                                lhsT=s2[:, ti * 128 : (ti + 1) * 128], rhs=rhs,
                                start=t not in started, stop=False,
                                skip_group_check=True)
                            started.add(t)
        # self-loop chunk per tile (tile's own rows, diagonal S), then finalize
        for jp, p in enumerate(pairs):
            for side in range(2):
                t = 2 * p + side
                if t >= TILES:
                    continue
                ht = wp.tile([128, TW], F32, tag="ht")
                nc.scalar.dma_start(out=ht[:, 0:TW],
                                    in_=haug[l][t * 128 : (t + 1) * 128, :])
                exs = wp.tile([128, 2], F32, tag="exs")
                nc.vector.tensor_tensor(
                    out=exs[:, 0:H], in0=ht[:, L["ALS"] : L["ALS"] + H],
                    in1=ht[:, L["ALS"] + H : L["ALS"] + 2 * H], op=OP.add)
                exs2 = wp.tile([128, 2], F32, tag="exs2")
                nc.vector.tensor_scalar(out=exs2[:, 0:H], in0=exs[:, 0:H],
                                        scalar1=0.2, scalar2=None, op0=OP.mult)
                nc.vector.tensor_tensor(out=exs[:, 0:H], in0=exs[:, 0:H],
                                        in1=exs2[:, 0:H], op=OP.max)
                nc.scalar.activation(exs[:, 0:H], exs[:, 0:H], AF.Exp)
                for h in range(H):
                    ss = wp.tile([128, 128], F32, tag="ss")
                    nc.vector.tensor_scalar(
                        out=ss[:], in0=R["iota"][:, 0:128],
                        scalar1=R["pidx"][:, 0:1],
                        scalar2=exs[:, h : h + 1],
                        op0=OP.is_equal, op1=OP.mult)
                    if l == 0:
                        rhs = ht[:, h * 129 : h * 129 + RW]
                    else:
                        rhs = ht[:, 0:RW]
                    nc.tensor.matmul(
                        ptile[t][:, h * RW : (h + 1) * RW], lhsT=ss[:], rhs=rhs,
                        start=t not in started, stop=h == H - 1,
                        skip_group_check=True)
                    started.add(t)
                finalize_tile(nc, cfg, l, t, ptile[t], R, out, haug, wp, pt, pz)
        call16 += NR * ns // 16
        ccol0 += NR * nb


def finalize_tile(nc, cfg, l, t, ps, R, out, haug, wp, pt, pz):
    L = LAYERS[l]
    H, RW = L["H"], L["RW"]
    rows = slice(t * 128, (t + 1) * 128)
    if l < 2:
        rc = wp.tile([128, 2], F32, tag="rc")
        if l == 0:  # den at cols 128 and 257
            nc.vector.reciprocal(rc[:, 0:1], ps[:, 128:129])
            nc.vector.reciprocal(rc[:, 1:2], ps[:, 257:258])
        else:
            nc.vector.reciprocal(rc[:, 0:1], ps[:, 256:257])
        z = wp.tile([128, 256], F32, tag="z")
        if l == 0:
            nc.vector.tensor_scalar(out=z[:, 0:128], in0=ps[:, 0:128],
                                    scalar1=rc[:, 0:1], scalar2=None, op0=OP.mult)
            nc.vector.tensor_scalar(out=z[:, 128:256], in0=ps[:, 129:257],
                                    scalar1=rc[:, 1:2], scalar2=None, op0=OP.mult)
        else:
            nc.vector.tensor_scalar(out=z[:], in0=ps[:, 0:256],
                                    scalar1=rc[:, 0:1], scalar2=None, op0=OP.mult)
        A, B = ("A1", "B1") if l == 0 else ("A2", "B2")
        nc.vector.tensor_tensor(out=z[:], in0=z[:], in1=R[A][:], op=OP.mult)
        nc.vector.tensor_tensor(out=z[:], in0=z[:], in1=R[B][:], op=OP.add)
        nc.scalar.activation(z[:], z[:], AF.Relu)
        # fused next-layer projection: haug[l+1][t] = z @ w_next
        zt_ps = pt.tile([128, 256], F32, tag="zt")
        for k in range(2):
            nc.tensor.transpose(zt_ps[:, k * 128 : (k + 1) * 128],
                                z[:, k * 128 : (k + 1) * 128], R["ident"][:])
        zt = wp.tile([128, 256], F32, tag="zts")
        nc.scalar.activation(zt[:], zt_ps[:], AF.Copy)
        wn, w = ("w2", 320) if l == 0 else ("w3", 64)
        pp = pz.tile([128, 320], F32, tag="proj")
        for k in range(2):
            nc.tensor.matmul(pp[:, 0:w], lhsT=zt[:, k * 128 : (k + 1) * 128],
                             rhs=R[wn][:, k * w : (k + 1) * w],
                             start=k == 0, stop=k == 1)
        hs = wp.tile([128, 320], F32, tag="hs")
        nc.scalar.activation(hs[:, 0:w], pp[:, 0:w], AF.Copy)
        if l == 0:
            nc.vector.memset(hs[:, 256:257], 1.0)
        else:
            nc.vector.memset(hs[:, 40:41], 1.0)
        nc.scalar.dma_start(out=haug[l + 1][rows, :], in_=hs[:, 0:w])
    else:
        # out = feats/den + b3, then log_softmax
        rc = wp.tile([128, 2], F32, tag="rc")
        nc.vector.reciprocal(rc[:, 0:1], ps[:, 40:41])
        o = wp.tile([128, 40], F32, tag="o")
        nc.vector.tensor_scalar(out=o[:], in0=ps[:, 0:40], scalar1=rc[:, 0:1],
                                scalar2=None, op0=OP.mult)
        nc.vector.tensor_tensor(out=o[:], in0=o[:], in1=R["b3"][:], op=OP.add)
        nmx = wp.tile([128, 1], F32, tag="nmx")
        nc.vector.tensor_reduce(out=nmx[:], in_=o[:], op=OP.max,
                                axis=mybir.AxisListType.X, negate=True)
        tmp = wp.tile([128, 40], F32, tag="tmp")
        se = wp.tile([128, 1], F32, tag="se")
        nc.scalar.activation(tmp[:], o[:], AF.Exp, bias=nmx[:, 0:1], accum_out=se[:])
        lse = wp.tile([128, 1], F32, tag="lse")
        nc.scalar.activation(lse[:], se[:], AF.Ln)
        o2 = wp.tile([128, 40], F32, tag="o2")
        nc.vector.tensor_scalar(out=o2[:], in0=o[:], scalar1=nmx[:, 0:1],
                                scalar2=lse[:, 0:1], op0=OP.add, op1=OP.subtract)
        nc.scalar.dma_start(out=out[t * 128 : (t + 1) * 128, :], in_=o2[:])


# ------------------------------------------------------------------ entry


_CACHE = {}


def kernel(**inputs):
    return kernel_cfg(make_cfg(169343, 166, 6), **inputs)


def kernel_cfg(cfg, **inputs):
    x = np.asarray(inputs["x"], np.float32)
    src = np.asarray(inputs["src"])
    dst = np.asarray(inputs["dst"])
    gidx, didx, dstloc, xT, perm = prepare(cfg, x, src, dst)
    W = prep_weights({k: np.asarray(v) for k, v in inputs.items()})
    key = cfg["NSH"]
    if key not in _CACHE:
        _CACHE[key] = build(cfg)
    nc = _CACHE[key]
    in_maps = []
    for c in range(NCORES):
        m = dict(W)
        m["xT"] = xT[c]
        m["gidx"] = gidx[c]
        m["didx"] = didx[c]
        m["dstloc"] = dstloc[c]
        in_maps.append(m)
    res = run_bass_kernel_spmd(nc, in_maps, core_ids=list(range(NCORES)))
    big = np.concatenate([res.results[c]["out"] for c in range(NCORES)], 0)
    return big[perm[: cfg["N"]]].astype(np.float32)


# revision 15
# speedup vs baseline: 1.1127x; 1.1127x over previous
"""3-layer GAT (ogbn-arxiv shapes) on 8 Trainium2 NeuronCores.

Graph/data-parallel per the sharding hint: nodes are sharded contiguously
across the 8 cores.  Per layer each core projects its shard with an augmented
weight matrix (attention vectors ride as extra columns), AllGathers the
augmented node table, then aggregates its incoming edges: int16 dma_gather of
source rows (6 static 32768-row ranges of the table), per-edge softmax weights
on ACT/DVE, and segment-sums via one-hot matrices on the tensor engine into
per-tile PSUM accumulators.  BN+ReLU and the next layer's projection are fused
into the per-tile finalize; log_softmax at the end.

Edge->slot layout is computed on the host with a static chunk->tile-pair
assignment so the SPMD program is identical on every core.
"""

import time

import numpy as np

import concourse.bacc as bacc
import concourse.bass as bass
import concourse.mybir as mybir
import concourse.tile as tile
from concourse.bass_utils import run_bass_kernel_spmd

F32 = mybir.dt.float32
I16 = mybir.dt.int16
AF = mybir.ActivationFunctionType
OP = mybir.AluOpType

NCORES = 8

# L1 row: [h0f 128 | one | h1f 128 | one | als0 als1 | ald0 ald1 | 0...] w=320
# L2 row: [feats 256 | one | als | ald | 0...] w=320
# L3 row: [feats 40 | one | als | ald | 0...] w=64
LAYERS = [
    dict(TW=320, H=2, RW=129, ALS=258, ATT0=256, ALDB=4),
    dict(TW=320, H=1, RW=257, ALS=257, ATT0=256, ALDB=2),
    dict(TW=64, H=1, RW=41, ALS=41, ATT0=0, ALDB=42),
]


def make_cfg(n_nodes, tiles_per_core, n_ranges):
    nsh = tiles_per_core * 128
    cfg = dict(N=n_nodes, NSH=nsh, TILES=tiles_per_core,
               PAIRS=(tiles_per_core + 1) // 2, NR=n_ranges, NTOT=nsh * NCORES)
    cfg["RANGE"] = -(-cfg["NTOT"] // n_ranges)
    assert cfg["RANGE"] <= 32768
    cfg["GROUPS"] = -(-cfg["PAIRS"] // 3)
    return cfg


def group_pairs(cfg, g):
    return [p for p in range(3 * g, 3 * g + 3) if p < cfg["PAIRS"]]


def cfg_slots(cfg):
    return sum(cfg["NR"] * 256 * len(group_pairs(cfg, g)) for g in range(cfg["GROUPS"]))


# ------------------------------------------------------------------ host prep


def prepare(cfg, x, src, dst):
    N, NSH, NTOT, NR = cfg["N"], cfg["NSH"], cfg["NTOT"], cfg["NR"]
    s = src.astype(np.int64)
    d = dst.astype(np.int64)

    perm = np.arange(NTOT, dtype=np.int64)  # node -> row
    for _ in range(30):
        srow, drow = perm[s], perm[d]
        key = ((drow // NSH) * cfg["PAIRS"] + (drow % NSH) // 256) * NR + srow // cfg["RANGE"]
        cnt = np.bincount(key, minlength=NCORES * cfg["PAIRS"] * NR)
        over = np.nonzero(cnt > 256)[0]
        if len(over) == 0:
            break
        inv = np.empty(NTOT, np.int64)
        inv[perm] = np.arange(NTOT)
        cnt3 = cnt.reshape(NCORES, cfg["PAIRS"], NR)
        load = cnt3.max(axis=2)
        for k in over[:64]:
            c = k // (cfg["PAIRS"] * NR)
            p = (k // NR) % cfg["PAIRS"]
            p2 = int(np.argmin(load[c]))
            a = inv[c * NSH + p * 256]
            b = inv[c * NSH + p2 * 256]
            perm[a], perm[b] = perm[b], perm[a]
            load[c, p2] += 8
    else:
        raise RuntimeError("pair balancing failed")

    srow, drow = perm[s], perm[d]
    core = drow // NSH
    pair = (drow % NSH) // 256
    rng = srow // cfg["RANGE"]

    G = cfg["GROUPS"]
    blk_off = np.zeros(cfg["PAIRS"], np.int64)
    call_off = np.zeros((G, NR), np.int64)
    off = 0
    for g in range(G):
        bp = group_pairs(cfg, g)
        for j, p in enumerate(bp):
            blk_off[p] = 256 * j
        for r in range(NR):
            call_off[g, r] = off
            off += 256 * len(bp)
    slots = off
    assert slots == cfg_slots(cfg)

    gidx = np.zeros((NCORES, slots), np.int16)
    didx = np.zeros((NCORES, slots), np.int16)
    dstloc = np.full((NCORES, slots), -1.0, np.float32)

    order = np.lexsort((drow, rng, pair, core))
    so, do_, co, po, ro = srow[order], drow[order], core[order], pair[order], rng[order]
    keyo = (co * cfg["PAIRS"] + po) * NR + ro
    chg = np.empty(len(keyo), bool)
    if len(keyo) == 0:
        chg = chg
    else:
        chg[0] = True
    chg[1:] = keyo[1:] != keyo[:-1]
    if len(keyo):
        sidx = np.nonzero(chg)[0]
        pos = np.arange(len(keyo)) - np.repeat(sidx, np.diff(np.append(sidx, len(keyo))))
        assert pos.max() < 256
        sl = call_off[po // 3, ro] + blk_off[po] + pos
        gidx[co, sl] = (so - ro * cfg["RANGE"]).astype(np.int16)
        didx[co, sl] = (do_ - co * NSH).astype(np.int16)
        dstloc[co, sl] = (do_ - (co * NSH + po * 256)).astype(np.float32)

    def wrap(a):  # index i -> [i%16, i//16], replicated to 128 partitions
        ncol = a.shape[1] // 16
        w = a.reshape(a.shape[0], ncol, 16).transpose(0, 2, 1)
        return np.ascontiguousarray(np.tile(w, (1, 8, 1)))

    nch = slots // 128
    dstloc_c = np.ascontiguousarray(dstloc.reshape(NCORES, nch, 128).transpose(0, 2, 1))

    inv = np.empty(NTOT, np.int64)
    inv[perm] = np.arange(NTOT)
    xsh = np.zeros((NCORES, NSH, x.shape[1]), np.float32)
    for c in range(NCORES):
        rows = inv[c * NSH : (c + 1) * NSH]
        real = rows < N
        xsh[c][real] = x[rows[real]]
    xT = np.ascontiguousarray(xsh.transpose(0, 2, 1))
    return wrap(gidx), wrap(didx), dstloc_c, xT, perm


def prep_weights(inp):
    EPS = 1e-5
    HID = inp["as1"].shape[1]
    w1 = np.zeros((inp["W1"].shape[0], 320), np.float32)
    w1[:, 0:128] = inp["W1"][:, :HID]
    w1[:, 129:257] = inp["W1"][:, HID:]
    w1[:, 258] = inp["W1"][:, :HID] @ inp["as1"][0]
    w1[:, 259] = inp["W1"][:, HID:] @ inp["as1"][1]
    w1[:, 260] = inp["W1"][:, :HID] @ inp["ad1"][0]
    w1[:, 261] = inp["W1"][:, HID:] @ inp["ad1"][1]
    w2 = np.zeros((256, 320), np.float32)
    w2[:, 0:256] = inp["W2"]
    w2[:, 257] = inp["W2"] @ inp["as2"][0]
    w2[:, 258] = inp["W2"] @ inp["ad2"][0]
    w3 = np.zeros((256, 64), np.float32)
    w3[:, 0:40] = inp["W3"]
    w3[:, 41] = inp["W3"] @ inp["as3"][0]
    w3[:, 42] = inp["W3"] @ inp["ad3"][0]

    def fold(b, g, be, m, v):
        k = g / np.sqrt(v + EPS)
        return k.astype(np.float32), ((b - m) * k + be).astype(np.float32)

    A1, B1 = fold(inp["b1"], inp["g1"], inp["be1"], inp["m1"], inp["v1"])
    A2, B2 = fold(inp["b2"], inp["g2"], inp["be2"], inp["m2"], inp["v2"])
    rep = lambda a: np.ascontiguousarray(np.tile(a[None, :], (128, 1)))
    return dict(w1=w1, w2=w2, w3=w3, A1=rep(A1), B1=rep(B1), A2=rep(A2),
                B2=rep(B2), b3=rep(inp["b3"].astype(np.float32)),
                iota=rep(np.arange(256, dtype=np.float32)),
                pidx=np.ascontiguousarray(
                    np.arange(128, dtype=np.float32)[:, None]),
                ident=np.eye(128, dtype=np.float32))


# ------------------------------------------------------------------ builder


def build(cfg):
    NSH, TILES, NR, G = cfg["NSH"], cfg["TILES"], cfg["NR"], cfg["GROUPS"]
    SLOTS = cfg_slots(cfg)
    nc = bacc.Bacc()
    ext = lambda n, sh, dt=F32: nc.dram_tensor(n, sh, dt, kind="ExternalInput")
    D = dict(
        xT=ext("xT", [128, NSH]), w1=ext("w1", [128, 320]),
        w2=ext("w2", [256, 320]), w3=ext("w3", [256, 64]),
        A1=ext("A1", [128, 256]), B1=ext("B1", [128, 256]),
        A2=ext("A2", [128, 256]), B2=ext("B2", [128, 256]),
        b3=ext("b3", [128, 40]), iota=ext("iota", [128, 256]),
        pidx=ext("pidx", [128, 1]),
        ident=ext("ident", [128, 128]),
        gidx=ext("gidx", [128, SLOTS // 16], I16),
        didx=ext("didx", [128, SLOTS // 16], I16),
        dstloc=ext("dstloc", [128, SLOTS // 128]),
    )
    out = nc.dram_tensor("out", [NSH, 40], F32, kind="ExternalOutput")
    dbgG = nc.dram_tensor("dbgG", [128, 6 * 320], F32, kind="ExternalOutput")
    haug = [nc.dram_tensor(f"haug{l}", [NSH, LAYERS[l]["TW"]], F32) for l in range(3)]
    tabs = [nc.dram_tensor(f"tab{l}", [cfg["NTOT"], LAYERS[l]["TW"]], F32,
                           addr_space="Shared") for l in range(3)]

    with tile.TileContext(nc) as tc:
        with (
            tc.tile_pool(name="res", bufs=1) as res,
            tc.tile_pool(name="gp", bufs=2) as gp,
            tc.tile_pool(name="wp", bufs=3) as wp,
            tc.tile_pool(name="pt", bufs=1, space="PSUM") as pt,
            tc.tile_pool(name="pz", bufs=1, space="PSUM") as pz,
            tc.tile_pool(name="pagg", bufs=1, space="PSUM") as pagg,
        ):
            R = {}
            for n, sh, dt in (
                ("w1", [128, 320], F32), ("w2", [128, 640], F32),
                ("w3", [128, 128], F32), ("A1", [128, 256], F32),
                ("B1", [128, 256], F32), ("A2", [128, 256], F32),
                ("B2", [128, 256], F32), ("b3", [128, 40], F32),
                ("iota", [128, 256], F32), ("ident", [128, 128], F32),
                ("pidx", [128, 1], F32),
                ("gidx", [128, SLOTS // 16], I16),
                ("didx", [128, SLOTS // 16], I16),
                ("dstloc", [128, SLOTS // 128], F32),
            ):
                R[n] = res.tile(sh, dt, name=n, tag=n)
                if n in ("w2", "w3"):
                    w = sh[1] // 2
                    for k in range(2):
                        nc.scalar.dma_start(
                            out=R[n][:, k * w : (k + 1) * w],
                            in_=D[n][k * 128 : (k + 1) * 128, :])
                else:
                    nc.scalar.dma_start(out=R[n][:], in_=D[n][:])

            # ---- layer-1 projection
            for t in range(TILES):
                xt = wp.tile([128, 128], F32, tag="xt")
                nc.scalar.dma_start(out=xt[:], in_=D["xT"][:, t * 128 : (t + 1) * 128])
                ps = pz.tile([128, 320], F32, tag="proj")
                nc.tensor.matmul(ps[:], lhsT=xt[:], rhs=R["w1"][:], start=True, stop=True)
                hs = wp.tile([128, 320], F32, tag="hs")
                nc.scalar.activation(hs[:], ps[:], AF.Copy)
                nc.vector.memset(hs[:, 128:129], 1.0)
                nc.vector.memset(hs[:, 257:258], 1.0)
                nc.scalar.dma_start(out=haug[0][t * 128 : (t + 1) * 128, :], in_=hs[:])

            for l in range(3):
                nc.gpsimd.collective_compute(
                    "AllGather", OP.bypass,
                    ins=[haug[l][:].opt()], outs=[tabs[l][:].opt()],
                    replica_groups=[list(range(NCORES))])
                edge_phase(nc, cfg, l, R, out, haug, tabs, gp, wp, pt, pz, pagg, dbgG)
    nc.compile()
    return nc


def edge_phase(nc, cfg, l, R, out, haug, tabs, gp, wp, pt, pz, pagg, dbgG=None):
    L = LAYERS[l]
    TW, H, RW = L["TW"], L["H"], L["RW"]
    NR, G, TILES = cfg["NR"], cfg["GROUPS"], cfg["TILES"]
    PW = H * RW  # psum width used
    call16 = 0
    ccol0 = 0
    for g in range(G):
        pairs = group_pairs(cfg, g)
        ns = 256 * len(pairs)
        nb = ns // 128
        # gathers
        Gt = []
        for r in range(NR):
            gt = gp.tile([128, 6 * TW], F32, tag=f"G{r}")
            lo = r * cfg["RANGE"]
            hi = min(lo + cfg["RANGE"], cfg["NTOT"])
            nc.gpsimd.dma_gather(
                out_ap=gt[:, : nb * TW].rearrange("p (b t) -> p b t", b=nb),
                in_ap=tabs[l][lo:hi, :],
                idxs_ap=R["gidx"][:, call16 + r * (ns // 16) : call16 + (r + 1) * (ns // 16)],
                num_idxs=ns, num_idxs_reg=ns, elem_size=TW, single_packet=False)
            if l == 0 and g == 0 and r == 0 and dbgG is not None:
                nc.scalar.dma_start(out=dbgG[:, : nb * TW], in_=gt[:, : nb * TW])
            Gt.append(gt)
        ad = gp.tile([128, 6 * NR * 64], F32, tag="ald")
        nc.gpsimd.dma_gather(
            out_ap=ad[:, : NR * nb * 64].rearrange("p (b t) -> p b t", b=NR * nb),
            in_ap=haug[l][:, L["ATT0"] : L["ATT0"] + 64],
            idxs_ap=R["didx"][:, call16 : call16 + NR * (ns // 16)],
            num_idxs=NR * ns, num_idxs_reg=NR * ns, elem_size=64,
            elem_step=TW, single_packet=False)
        # per-edge weights exp(lrelu(als[src] + ald[dst]))
        exw = []
        for r in range(NR):
            ex = wp.tile([128, 12], F32, tag=f"ex{r}")
            gv = Gt[r][:, : nb * TW].rearrange("p (b t) -> p b t", b=nb)
            av = ad[:, : NR * nb * 64].rearrange("p (b t) -> p b t", b=NR * nb)
            ev = ex[:, : nb * H].rearrange("p (b t) -> p b t", b=nb)
            nc.vector.tensor_tensor(
                out=ev, in0=gv[:, :, L["ALS"] : L["ALS"] + H],
                in1=av[:, r * nb : (r + 1) * nb, L["ALDB"] : L["ALDB"] + H],
                op=OP.add)
            ex2 = wp.tile([128, 12], F32, tag=f"ex2_{r}")
            nc.vector.tensor_scalar(out=ex2[:, : nb * H], in0=ex[:, : nb * H],
                                    scalar1=0.2, scalar2=None, op0=OP.mult)
            nc.vector.tensor_tensor(out=ex[:, : nb * H], in0=ex[:, : nb * H],
                                    in1=ex2[:, : nb * H], op=OP.max)
            nc.scalar.activation(ex[:, : nb * H], ex[:, : nb * H], AF.Exp)
            if l == 0 and g == 0 and r == 0 and dbgG is not None:
                nc.scalar.dma_start(out=dbgG[:, 1600 : 1600 + nb * H], in_=ex[:, : nb * H])
            exw.append(ex)
        # chunk matmuls into per-tile psums
        ptile = {}
        for jp, p in enumerate(pairs):
            for side in range(2):
                t = 2 * p + side
                if t < TILES:
                    ptile[t] = pagg.tile([128, 272], F32, name=f"agg_t{t}", tag=f"agg{t % 6}")
        started = set()
        for jp, p in enumerate(pairs):
            for r in range(NR):
                for side in range(2):
                    b = 2 * jp + side
                    ccol = ccol0 + r * nb + b
                    for h in range(H):
                        s2 = wp.tile([128, 256], F32, tag="s2")
                        nc.vector.tensor_scalar(
                            out=s2[:], in0=R["iota"][:],
                            scalar1=R["dstloc"][:, ccol : ccol + 1],
                            scalar2=exw[r][:, b * H + h : b * H + h + 1],
                            op0=OP.is_equal, op1=OP.mult)
                        if l == 0:
                            rhs = Gt[r][:, b * TW + h * 129 : b * TW + h * 129 + RW]
                        else:
                            rhs = Gt[r][:, b * TW : b * TW + RW]
                        for ti in range(2):
                            t = 2 * p + ti
                            if t >= TILES:
                                continue
                            nc.tensor.matmul(
                                ptile[t][:, h * RW : (h + 1) * RW],
                                lhsT=s2[:, ti * 128 : (ti + 1) * 128], rhs=rhs,
                                start=t not in started, stop=False,
                                skip_group_check=True)
                            started.add(t)
        # self-loop chunk per tile (tile's own rows, diagonal S), then finalize
        for jp, p in enumerate(pairs):
            for side in range(2):
                t = 2 * p + side
                if t >= TILES:
                    continue
                ht = wp.tile([128, TW], F32, tag="ht")
                nc.scalar.dma_start(out=ht[:, 0:TW],
                                    in_=haug[l][t * 128 : (t + 1) * 128, :])
                exs = wp.tile([128, 2], F32, tag="exs")
                nc.vector.tensor_tensor(
                    out=exs[:, 0:H], in0=ht[:, L["ALS"] : L["ALS"] + H],
                    in1=ht[:, L["ALS"] + H : L["ALS"] + 2 * H], op=OP.add)
                exs2 = wp.tile([128, 2], F32, tag="exs2")
                nc.vector.tensor_scalar(out=exs2[:, 0:H], in0=exs[:, 0:H],
                                        scalar1=0.2, scalar2=None, op0=OP.mult)
                nc.vector.tensor_tensor(out=exs[:, 0:H], in0=exs[:, 0:H],
                                        in1=exs2[:, 0:H], op=OP.max)
                nc.scalar.activation(exs[:, 0:H], exs[:, 0:H], AF.Exp)
                for h in range(H):
                    ss = wp.tile([128, 128], F32, tag="ss")
                    nc.vector.tensor_scalar(
                        out=ss[:], in0=R["iota"][:, 0:128],
                        scalar1=R["pidx"][:, 0:1],
                        scalar2=exs[:, h : h + 1],
                        op0=OP.is_equal, op1=OP.mult)
                    if l == 0:
                        rhs = ht[:, h * 129 : h * 129 + RW]
                    else:
                        rhs = ht[:, 0:RW]
                    nc.tensor.matmul(
                        ptile[t][:, h * RW : (h + 1) * RW], lhsT=ss[:], rhs=rhs,
                        start=t not in started, stop=h == H - 1,
                        skip_group_check=True)
                    started.add(t)
                finalize_tile(nc, cfg, l, t, ptile[t], R, out, haug, wp, pt, pz)
        call16 += NR * ns // 16
        ccol0 += NR * nb


def finalize_tile(nc, cfg, l, t, ps, R, out, haug, wp, pt, pz):
    L = LAYERS[l]
    H, RW = L["H"], L["RW"]
    rows = slice(t * 128, (t + 1) * 128)
    if l < 2:
        rc = wp.tile([128, 2], F32, tag="rc")
        if l == 0:  # den at cols 128 and 257
            nc.vector.reciprocal(rc[:, 0:1], ps[:, 128:129])
            nc.vector.reciprocal(rc[:, 1:2], ps[:, 257:258])
        else:
            nc.vector.reciprocal(rc[:, 0:1], ps[:, 256:257])
        z = wp.tile([128, 256], F32, tag="z")
        if l == 0:
            nc.vector.tensor_scalar(out=z[:, 0:128], in0=ps[:, 0:128],
                                    scalar1=rc[:, 0:1], scalar2=None, op0=OP.mult)
            nc.vector.tensor_scalar(out=z[:, 128:256], in0=ps[:, 129:257],
                                    scalar1=rc[:, 1:2], scalar2=None, op0=OP.mult)
        else:
            nc.vector.tensor_scalar(out=z[:], in0=ps[:, 0:256],
                                    scalar1=rc[:, 0:1], scalar2=None, op0=OP.mult)
        A, B = ("A1", "B1") if l == 0 else ("A2", "B2")
        nc.vector.tensor_tensor(out=z[:], in0=z[:], in1=R[A][:], op=OP.mult)
        nc.vector.tensor_tensor(out=z[:], in0=z[:], in1=R[B][:], op=OP.add)
        nc.scalar.activation(z[:], z[:], AF.Relu)
        # fused next-layer projection: haug[l+1][t] = z @ w_next
        zt_ps = pt.tile([128, 256], F32, tag="zt")
        for k in range(2):
            nc.tensor.transpose(zt_ps[:, k * 128 : (k + 1) * 128],
                                z[:, k * 128 : (k + 1) * 128], R["ident"][:])
        zt = wp.tile([128, 256], F32, tag="zts")
        nc.scalar.activation(zt[:], zt_ps[:], AF.Copy)
        wn, w = ("w2", 320) if l == 0 else ("w3", 64)
        pp = pz.tile([128, 320], F32, tag="proj")
        for k in range(2):
            nc.tensor.matmul(pp[:, 0:w], lhsT=zt[:, k * 128 : (k + 1) * 128],
                             rhs=R[wn][:, k * w : (k + 1) * w],
                             start=k == 0, stop=k == 1)
        hs = wp.tile([128, 320], F32, tag="hs")
        nc.scalar.activation(hs[:, 0:w], pp[:, 0:w], AF.Copy)
        if l == 0:
            nc.vector.memset(hs[:, 256:257], 1.0)
        else:
            nc.vector.memset(hs[:, 40:41], 1.0)
        nc.scalar.dma_start(out=haug[l + 1][rows, :], in_=hs[:, 0:w])
    else:
        # out = feats/den + b3, then log_softmax
        rc = wp.tile([128, 2], F32, tag="rc")
        nc.vector.reciprocal(rc[:, 0:1], ps[:, 40:41])
        o = wp.tile([128, 40], F32, tag="o")
        nc.vector.tensor_scalar(out=o[:], in0=ps[:, 0:40], scalar1=rc[:, 0:1],
                                scalar2=None, op0=OP.mult)
        nc.vector.tensor_tensor(out=o[:], in0=o[:], in1=R["b3"][:], op=OP.add)
        nmx = wp.tile([128, 1], F32, tag="nmx")
        nc.vector.tensor_reduce(out=nmx[:], in_=o[:], op=OP.max,
                                axis=mybir.AxisListType.X, negate=True)
        tmp = wp.tile([128, 40], F32, tag="tmp")
        se = wp.tile([128, 1], F32, tag="se")
        nc.scalar.activation(tmp[:], o[:], AF.Exp, bias=nmx[:, 0:1], accum_out=se[:])
        lse = wp.tile([128, 1], F32, tag="lse")
        nc.scalar.activation(lse[:], se[:], AF.Ln)
        o2 = wp.tile([128, 40], F32, tag="o2")
        nc.vector.tensor_scalar(out=o2[:], in0=o[:], scalar1=nmx[:, 0:1],
                                scalar2=lse[:, 0:1], op0=OP.add, op1=OP.subtract)
        nc.scalar.dma_start(out=out[t * 128 : (t + 1) * 128, :], in_=o2[:])


# ------------------------------------------------------------------ entry


_CACHE = {}
LAST_TIMES = []


def kernel(**inputs):
    return kernel_cfg(make_cfg(169343, 166, 6), **inputs)


def kernel_cfg(cfg, **inputs):
    x = np.asarray(inputs["x"], np.float32)
    src = np.asarray(inputs["src"])
    dst = np.asarray(inputs["dst"])
    gidx, didx, dstloc, xT, perm = prepare(cfg, x, src, dst)
    W = prep_weights({k: np.asarray(v) for k, v in inputs.items()})
    key = cfg["NSH"]
    if key not in _CACHE:
        _CACHE[key] = build(cfg)
    nc = _CACHE[key]
    in_maps = []
    for c in range(NCORES):
        m = dict(W)
        m["xT"] = xT[c]
        m["gidx"] = gidx[c]
        m["didx"] = didx[c]
        m["dstloc"] = dstloc[c]
        in_maps.append(m)
    t0 = time.time()
    res = run_bass_kernel_spmd(nc, in_maps, core_ids=list(range(NCORES)))
    LAST_TIMES.append(time.time() - t0)
    big = np.concatenate([res.results[c]["out"] for c in range(NCORES)], 0)
    return big[perm[: cfg["N"]]].astype(np.float32)
